# revision 1
# baseline (speedup 1.0000x reference)
"""TRN2 Bass kernel for nn_CBBIOMFP_9234179686614 (topk_masking).

Reference computation (B=32, S=512, D=128, V=25, H=2):
  x = emb[peptide] + pe
  a-encoder (1 layer; only QK needed) -> attention col-sums -> mask [B,S]
  creat_mask: per row drop the ceil(0.2*j) smallest mask values before the
  first pad (token==0) position j -> mask_pep
  h  = e-encoder(3 layers)(x),  attn_score = layer-3 attention probs
  hm = t-encoder(2 layers)(emb[mask_pep]+pe)
  z1 = head(h.flat), z2 = head(hm.flat)   (65536->1024->512->128 MLP)
  returns (h1, z1, h2, z2, attn_score)

Sharding: data-parallel over batch for the encoders (4 seqs/core); the big
65536x1024 head matmul is k-sharded (each core owns 8192 contraction rows
of p1_w), using an on-device AllToAll to redistribute h and an AllReduce
for the layer-1 partials.  Matmuls use float32r where the moving dim is
>=256; the top-k ranking path is kept in exact fp32.
"""
import sys
import numpy as np

sys.path.insert(0, "/opt/trn_rl_repo")

import concourse.bass as bass
import concourse.mybir as mybir
import concourse.tile as tile
from concourse import bacc
from concourse.bass_utils import run_bass_kernel_spmd

F32 = mybir.dt.float32
F32R = mybir.dt.float32r
EXP = mybir.ActivationFunctionType.Exp
RELU = mybir.ActivationFunctionType.Relu
SQRT = mybir.ActivationFunctionType.Sqrt
IDENT = mybir.ActivationFunctionType.Identity
OP = mybir.AluOpType
AX = mybir.AxisListType

B, S, D, V, H = 32, 512, 128, 25, 2
DK = D // H          # 64
DFF = 4 * D          # 512
NC = 8               # cores
BL = B // NC         # 4 seqs per core
ST = S // 128        # 4 s-tiles per seq
NT = BL * ST         # 16 s-tiles per core
SW = BL * S          # 2048 tokens per core
KSH = S // NC * D    # 8192 head contraction rows per core
PRECET = float(np.float32(0.2))

_cached = {}


def build():
    nc = bacc.Bacc(None, num_devices=NC)

    din = {}

    def inp(name, shape, dt=F32R):
        din[name] = nc.dram_tensor(name, list(shape), dt, kind="ExternalInput")
        return din[name]

    inp("pep_f", (BL, S), F32)
    inp("emb", (V, D))
    inp("a_wq", (D, D)); inp("a_wk", (D, D))
    for p, L in (("e", 3), ("t", 2)):
        inp(p + "_wq", (L, D, D)); inp(p + "_wk", (L, D, D))
        inp(p + "_wv", (L, D, D)); inp(p + "_wo", (L, D, D))
        inp(p + "_f1", (L, D, DFF)); inp(p + "_f2", (L, DFF, D))
    inp("p1s", (KSH, 1024))
    inp("p1b", (128, 8), F32)
    inp("p2w", (1024, 512))
    inp("p2b", (128, 4), F32)
    inp("p3w", (512, D))
    inp("p3b", (1, D))
    inp("peT", (D, S), F32)
    inp("ident32", (128, 128), F32)
    inp("identr", (128, 128))
    inp("iota_v", (128, V), F32)
    inp("iota_row", (1, S), F32)
    inp("iota_part", (128, ST), F32)
    inp("ones_r", (128, 128))

    outs = {
        "h1p": nc.dram_tensor("h1p", [BL, S * D], F32, kind="ExternalOutput"),
        "h2p": nc.dram_tensor("h2p", [BL, S * D], F32, kind="ExternalOutput"),
        "attnp": nc.dram_tensor("attnp", [BL, H, S, S], F32, kind="ExternalOutput"),
        "z1p": nc.dram_tensor("z1p", [B, D], F32, kind="ExternalOutput"),
        "z2p": nc.dram_tensor("z2p", [B, D], F32, kind="ExternalOutput"),
        "maskdbg": nc.dram_tensor("maskdbg", [BL, S], F32, kind="ExternalOutput"),
        "tokdbg": nc.dram_tensor("tokdbg", [BL, S], F32, kind="ExternalOutput"),
    }

    with tile.TileContext(nc) as tc:
        Body(nc, tc, din, outs).run()
    nc.compile()
    return nc


class Body:
    def __init__(self, nc, tc, din, outs):
        self.nc, self.tc, self.din, self.outs = nc, tc, din, outs

    def run(self):
        nc, tc, din = self.nc, self.tc, self.din
        import contextlib
        with contextlib.ExitStack() as es:
            self.sb = es.enter_context(tc.tile_pool(name="sb", bufs=1))
            self.ps = es.enter_context(tc.tile_pool(name="ps", bufs=1, space="PSUM"))
            self.dram = es.enter_context(tc.tile_pool(name="dram", bufs=1, space="DRAM"))
            self._body()

    def sbt(self, shape, dt=F32R, tag=None, bufs=1):
        return self.sb.tile(list(shape), dt, tag=tag, bufs=bufs)

    def pst(self, shape, tag, bufs=2):
        return self.ps.tile(list(shape), F32, tag=tag, bufs=bufs)

    def _body(self):
        nc, din, outs = self.nc, self.din, self.outs
        sbt, pst = self.sbt, self.pst
        cp = nc.vector.tensor_copy

        # ---- constants ----
        emb_sb = sbt((V, D), tag="c_emb")
        nc.sync.dma_start(emb_sb[:], din["emb"][:])
        peT = sbt((D, S), F32, tag="c_peT")
        nc.sync.dma_start(peT[:], din["peT"][:])
        id32 = sbt((128, 128), F32, tag="c_id32")
        nc.sync.dma_start(id32[:], din["ident32"][:])
        idr = sbt((128, 128), tag="c_idr")
        nc.sync.dma_start(idr[:], din["identr"][:])
        iov = sbt((128, V), F32, tag="c_iov")
        nc.sync.dma_start(iov[:], din["iota_v"][:])
        ior = sbt((1, S), F32, tag="c_ior")
        nc.sync.dma_start(ior[:], din["iota_row"][:])
        iop = sbt((128, ST), F32, tag="c_iop")
        nc.sync.dma_start(iop[:], din["iota_part"][:])
        onesr = sbt((128, 128), tag="c_onesr")
        nc.sync.dma_start(onesr[:], din["ones_r"][:])
        pep = sbt((BL, S), F32, tag="c_pep")
        nc.sync.dma_start(pep[:], din["pep_f"][:])
        self.idr, self.id32, self.onesr = idr, id32, onesr

        # ---- embedding of original tokens ----
        tokp = sbt((128, NT), F32, tag="tokp")
        padbias = sbt((128, NT), F32, tag="padb")
        padmask = sbt((128, BL, S), F32, tag="padm")
        ohT = sbt((V, SW), tag="ohT", bufs=2)
        x_ds = sbt((D, SW), tag="xds", bufs=3)
        x_sd = sbt((128, NT, D), tag="xsd", bufs=3)

        def embed(tok_col_src, ohT_o, xds_o, xsd_o, padb_o):
            for b in range(BL):
                for t in range(ST):
                    g = b * ST + t
                    tcol = tok_col_src(b, t, g)
                    nc.vector.tensor_scalar(padb_o[:, g:g + 1], tcol, 0.0, -1e9,
                                            OP.is_equal, OP.mult)
                    oh = sbt((128, V), tag="oh", bufs=3)
                    nc.vector.tensor_scalar(oh[:], iov[:], tcol, None, OP.is_equal)
                    poh = pst((V, 128), tag="p1c")
                    nc.tensor.transpose(poh[:], oh[:], idr[:])
                    cp(ohT_o[:, b * S + t * 128:b * S + (t + 1) * 128], poh[:])
                px = pst((D, S), tag="p512")
                nc.tensor.matmul(px[:], emb_sb[:], ohT_o[:, b * S:(b + 1) * S],
                                 start=True, stop=True)
                nc.vector.tensor_tensor(xds_o[:, b * S:(b + 1) * S], px[:], peT[:], OP.add)
                for t in range(ST):
                    pxs = pst((128, D), tag="p128")
                    nc.tensor.transpose(
                        pxs[:], xds_o[:, b * S + t * 128:b * S + (t + 1) * 128], idr[:])
                    cp(xsd_o[:, b * ST + t, :], pxs[:])

        def tok_from_pep(b, t, g):
            ptok = pst((128, 1), tag="p1c")
            nc.tensor.transpose(ptok[:], pep[b:b + 1, t * 128:(t + 1) * 128]
                                .bitcast(F32R), idr[:])
            cp(tokp[:, g:g + 1], ptok[:])
            return tokp[:, g:g + 1]

        embed(tok_from_pep, ohT, x_ds, x_sd, padbias)

        # pad masks broadcast across partitions (exact fp32 copy)
        padrow = sbt((1, BL, S), F32, tag="padrow")
        for b in range(BL):
            nc.vector.tensor_scalar(padrow[:, b, :], pep[b:b + 1, :], 0.0, 1.0,
                                    OP.not_equal, OP.mult)
            nc.gpsimd.partition_broadcast(padmask[:, b, :], padrow[:, b, :])

        # ---- a-encoder: E -> col-sums -> mask ----
        awq = sbt((D, D), tag="wq", bufs=2)
        nc.sync.dma_start(awq[:], din["a_wq"][:])
        awk = sbt((D, D), tag="wk", bufs=2)
        nc.sync.dma_start(awk[:], din["a_wk"][:])
        qT = sbt((D, SW), tag="qT", bufs=2)
        kT = sbt((D, SW), tag="kT", bufs=2)
        for c in range(ST):
            sl = slice(c * 512, (c + 1) * 512)
            pq = pst((D, 512), tag="p512")
            nc.tensor.matmul(pq[:], awq[:], x_ds[:, sl], start=True, stop=True)
            cp(qT[:, sl], pq[:])
            pk = pst((D, 512), tag="p512")
            nc.tensor.matmul(pk[:], awk[:], x_ds[:, sl], start=True, stop=True)
            cp(kT[:, sl], pk[:])

        mask_sb = sbt((1, BL, S), F32, tag="maskrow")
        for b in range(BL):
            pmask = pst((1, S), tag="prow", bufs=1)
            n_mm = 0
            for h in range(H):
                hd = slice(h * DK, (h + 1) * DK)
                for qt in range(ST):
                    psc = pst((128, S), tag="p512")
                    nc.tensor.matmul(
                        psc[:], qT[hd, b * S + qt * 128:b * S + (qt + 1) * 128],
                        kT[hd, b * S:(b + 1) * S], start=True, stop=True)
                    e_raw = sbt((128, S), F32, tag="Eraw", bufs=2)
                    nc.scalar.activation(e_raw[:], psc[:], EXP, bias=0.0, scale=0.125)
                    em = sbt((128, S), F32, tag="EM", bufs=2)
                    rs = sbt((128, 1), F32, tag="rs", bufs=8)
                    nc.vector.scalar_tensor_tensor(em[:], e_raw[:], 1.0,
                                                   padmask[:, b, :], OP.bypass,
                                                   OP.mult, accum_out=rs[:])
                    rsr = sbt((128, 1), F32, tag="rsr", bufs=8)
                    nc.vector.reciprocal(rsr[:], rs[:])
                    nc.tensor.matmul(pmask[:], rsr[:], em[:],
                                     start=(n_mm == 0), stop=(n_mm == 2 * ST - 1))
                    n_mm += 1
            cp(mask_sb[:, b, :], pmask[:])
            nc.sync.dma_start(outs["maskdbg"][b:b + 1, :], mask_sb[:, b, :])

        # ---- creat_mask + masked-token embedding ----
        tokp2 = sbt((128, NT), F32, tag="tokp2")
        padbias2 = sbt((128, NT), F32, tag="padb2")
        ohT2 = sbt((V, SW), tag="ohT", bufs=2)
        xm_ds = sbt((D, SW), tag="xmds")
        xm_sd = sbt((128, NT, D), tag="xmsd")

        for b in range(BL):
            mrow = mask_sb[:, b, :]
            is0 = sbt((1, S), F32, tag="cmrow", bufs=6)
            nc.vector.tensor_scalar(is0[:], pep[b:b + 1, :], 0.0, None, OP.is_equal)
            tmp = sbt((1, S), F32, tag="cmrow", bufs=6)
            nc.vector.scalar_tensor_tensor(tmp[:], is0[:], -1e9, ior[:], OP.mult, OP.add)
            jb = sbt((1, 1), F32, tag="cmsc", bufs=8)
            nc.vector.tensor_reduce(jb[:], tmp[:], AX.X, OP.min)
            h0 = sbt((1, 1), F32, tag="cmsc", bufs=8)
            nc.vector.tensor_scalar(h0[:], jb[:], 0.0, None, OP.is_lt)
            jv = sbt((1, 1), F32, tag="cmsc", bufs=8)
            nc.vector.scalar_tensor_tensor(jv[:], h0[:], 1e9, jb[:], OP.mult, OP.add)
            th = sbt((1, 1), F32, tag="cmsc", bufs=8)
            nc.vector.tensor_scalar(th[:], jv[:], PRECET, None, OP.mult)
            valid = sbt((1, S), F32, tag="cmrow", bufs=6)
            nc.vector.tensor_scalar(valid[:], ior[:], jv[:], None, OP.is_lt)
            mv = sbt((1, S), F32, tag="cmrow", bufs=6)
            nc.vector.scalar_tensor_tensor(mv[:], valid[:], -1e30, mrow, OP.mult, OP.add)
            mv2 = sbt((1, S), F32, tag="cmrow", bufs=6)
            nc.vector.tensor_scalar(mv2[:], mv[:], 1e30, None, OP.add)
            # exact fp32 partition broadcasts
            mvb = sbt((128, S), F32, tag="mvb")
            nc.gpsimd.partition_broadcast(mvb[:], mv2[:])
            tj = sbt((1, 2), F32, tag="cmsc2", bufs=2)
            cp(tj[:, 0:1], th[:])
            cp(tj[:, 1:2], jv[:])
            tjb = sbt((128, 2), F32, tag="tjb", bufs=2)
            nc.gpsimd.partition_broadcast(tjb[:], tj[:])
            for t in range(ST):
                g = b * ST + t
                pmp = pst((128, 1), tag="p1c")
                nc.tensor.transpose(pmp[:], mrow[:, t * 128:(t + 1) * 128], id32[:])
                mpart = sbt((128, 1), F32, tag="mpart", bufs=4)
                cp(mpart[:], pmp[:])
                scr = sbt((128, S), F32, tag="cmscr")
                cnt = sbt((128, 1), F32, tag="cnt", bufs=8)
                nc.vector.tensor_scalar(scr[:], mvb[:], mpart[:], None, OP.is_lt,
                                        accum_out=cnt[:])
                ge1 = sbt((128, 1), F32, tag="cnt", bufs=8)
                nc.vector.tensor_scalar(ge1[:], cnt[:], tjb[:, 0:1], None, OP.is_ge)
                keep = sbt((128, 1), F32, tag="cnt", bufs=8)
                nc.vector.scalar_tensor_tensor(keep[:], iop[:, t:t + 1], tjb[:, 1:2],
                                               ge1[:], OP.is_ge, OP.logical_or)
                nc.vector.tensor_tensor(tokp2[:, g:g + 1], tokp[:, g:g + 1], keep[:],
                                        OP.mult)
            nc.sync.dma_start(
                outs["tokdbg"][b:b + 1, :].rearrange("one (t p) -> p (one t)", p=128),
                tokp2[:, b * ST:(b + 1) * ST])

        def tok_from_masked(b, t, g):
            return tokp2[:, g:g + 1]

        embed(tok_from_masked, ohT2, xm_ds, xm_sd, padbias2)

        # ---- full encoder layers ----
        cc_in = self.dram.tile([NC, 2, BL, D, S // NC], F32)

        cur_ds, cur_sd = x_ds, x_sd
        for l in range(3):
            cur_ds, cur_sd = self.enc_layer(
                cur_ds, cur_sd, "e", l, padbias, padmask, emit_attn=(l == 2))
        self.emit_h(cur_sd, cur_ds, outs["h1p"], cc_in, 0)

        cur_ds, cur_sd = xm_ds, xm_sd
        for l in range(2):
            cur_ds, cur_sd = self.enc_layer(
                cur_ds, cur_sd, "t", l, padbias2, None, emit_attn=False)
        self.emit_h(cur_sd, cur_ds, outs["h2p"], cc_in, 1)

        # ---- AllToAll + head ----
        cc_out = self.dram.tile([NC, 2, BL, D, S // NC], F32)
        nc.gpsimd.collective_compute("AllToAll", OP.bypass,
                                     replica_groups=[list(range(NC))],
                                     ins=[cc_in.opt()], outs=[cc_out.opt()])

        py = pst((64, 1024), tag="phead", bufs=1)
        for sl in range(64):
            lh = sbt((128, 64), tag="hlhs", bufs=4)
            nc.sync.dma_start(
                lh[:].rearrange("d (e c b) -> d e c b", e=2, c=NC, b=BL),
                cc_out[:, :, :, :, sl].rearrange("c e b d -> d e c b"))
            w1 = sbt((128, 1024), tag="w1", bufs=3)
            nc.sync.dma_start(w1[:], din["p1s"][sl * 128:(sl + 1) * 128, :])
            nc.tensor.matmul(py[:, 0:512], lh[:].bitcast(F32R), w1[:, 0:512],
                             start=(sl == 0), stop=(sl == 63), skip_group_check=True)
            nc.tensor.matmul(py[:, 512:1024], lh[:].bitcast(F32R), w1[:, 512:1024],
                             start=(sl == 0), stop=(sl == 63), skip_group_check=True)
        y1_sb = sbt((64, 1024), F32, tag="y1sb", bufs=2)
        cp(y1_sb[:], py[:])
        y1in = self.dram.tile([64, 1024], F32)
        y1out = self.dram.tile([64, 1024], F32)
        nc.sync.dma_start(y1in[:], y1_sb[:])
        nc.gpsimd.collective_compute("AllReduce", OP.add,
                                     replica_groups=[list(range(NC))],
                                     ins=[y1in.opt()], outs=[y1out.opt()])
        y1f = sbt((64, 1024), F32, tag="y1sb", bufs=2)
        nc.sync.dma_start(y1f[:], y1out[:])

        p1b_sb = sbt((128, 8), F32, tag="p1b")
        nc.sync.dma_start(p1b_sb[:], din["p1b"][:])
        p2b_sb = sbt((128, 4), F32, tag="p2b")
        nc.sync.dma_start(p2b_sb[:], din["p2b"][:])
        p2w_sb = self.sb.tile([128, 8, 512], F32R, tag="y1T", bufs=2)
        nc.sync.dma_start(p2w_sb[:], din["p2w"][:].rearrange("(k p) n -> p k n", p=128))
        p3w_sb = sbt((128, 4, D), tag="p3w")
        nc.sync.dma_start(p3w_sb[:], din["p3w"][:].rearrange("(k p) n -> p k n", p=128))
        p3b_sb = sbt((1, D), tag="p3b")
        nc.sync.dma_start(p3b_sb[:], din["p3b"][:])

        y1T_h = sbt((128, 8, 64), tag="y1Th")
        for kt in range(8):
            pt = pst((128, 64), tag="p128")
            nc.tensor.transpose(pt[:], y1f[:, kt * 128:(kt + 1) * 128].bitcast(F32R),
                                idr[:])
            nc.scalar.activation(y1T_h[:, kt, :], pt[:], RELU,
                                 bias=p1b_sb[:, kt:kt + 1])
        pz2 = pst((64, 512), tag="phead", bufs=1)
        for kt in range(8):
            nc.tensor.matmul(pz2[:], y1T_h[:, kt, :], p2w_sb[:, kt, :],
                             start=(kt == 0), stop=(kt == 7))
        y2_sb = sbt((64, 512), F32, tag="y2sb")
        cp(y2_sb[:], pz2[:])
        y2T_h = sbt((128, 4, 64), tag="y2Th")
        for kt in range(4):
            pt = pst((128, 64), tag="p128")
            nc.tensor.transpose(pt[:], y2_sb[:, kt * 128:(kt + 1) * 128].bitcast(F32R),
                                idr[:])
            nc.scalar.activation(y2T_h[:, kt, :], pt[:], RELU,
                                 bias=p2b_sb[:, kt:kt + 1])
        pz3 = pst((64, D), tag="p128")
        for kt in range(4):
            nc.tensor.matmul(pz3[:], y2T_h[:, kt, :], p3w_sb[:, kt, :],
                             start=(kt == 0), stop=False)
        nc.tensor.matmul(pz3[:], onesr[0:1, 0:64], p3b_sb[:], start=False, stop=True)
        z_sb = sbt((64, D), F32, tag="zsb")
        cp(z_sb[:], pz3[:])
        nc.sync.dma_start(outs["z1p"][:], z_sb[0:32, :])
        nc.sync.dma_start(outs["z2p"][:], z_sb[32:64, :])

    # ------------------------------------------------------------------
    def enc_layer(self, x_ds_l, x_sd_l, pre, l, padb, padmask, emit_attn):
        nc, din = self.nc, self.din
        sbt, pst = self.sbt, self.pst
        cp = nc.vector.tensor_copy
        idr = self.idr

        wq = sbt((D, D), tag="wq", bufs=2)
        nc.sync.dma_start(wq[:], din[pre + "_wq"][l])
        wk = sbt((D, D), tag="wk", bufs=2)
        nc.sync.dma_start(wk[:], din[pre + "_wk"][l])
        wv = sbt((D, D), tag="wv", bufs=2)
        nc.sync.dma_start(wv[:], din[pre + "_wv"][l])
        wo = sbt((D, D), tag="wo", bufs=2)
        nc.sync.dma_start(wo[:], din[pre + "_wo"][l])
        f1 = sbt((D, DFF), tag="f1", bufs=2)
        nc.sync.dma_start(f1[:], din[pre + "_f1"][l])
        f2 = self.sb.tile([128, 4, D], F32R, tag="f2", bufs=2)
        nc.sync.dma_start(f2[:], din[pre + "_f2"][l].rearrange("(k p) d -> p k d", p=128))

        qT = sbt((D, SW), tag="qT", bufs=2)
        kT = sbt((D, SW), tag="kT", bufs=2)
        for c in range(ST):
            sl = slice(c * 512, (c + 1) * 512)
            pq = pst((D, 512), tag="p512")
            nc.tensor.matmul(pq[:], wq[:], x_ds_l[:, sl], start=True, stop=True)
            cp(qT[:, sl], pq[:])
            pk = pst((D, 512), tag="p512")
            nc.tensor.matmul(pk[:], wk[:], x_ds_l[:, sl], start=True, stop=True)
            cp(kT[:, sl], pk[:])

        vo = self.sb.tile([128, NT, H, DK + 1], F32R, tag="vones", bufs=1)
        nc.vector.memset(vo[:, :, :, DK:DK + 1], 1.0)
        for g in range(NT):
            pv = pst((128, D), tag="p128")
            nc.tensor.matmul(pv[:], x_ds_l[:, g * 128:(g + 1) * 128], wv[:],
                             start=True, stop=True)
            for h in range(H):
                cp(vo[:, g, h, 0:DK], pv[:, h * DK:(h + 1) * DK])

        ctx_sd = sbt((128, NT, D), tag="xsd", bufs=3)
        for b in range(BL):
            for h in range(H):
                hd = slice(h * DK, (h + 1) * DK)
                ET = []
                for kt in range(ST):
                    g = b * ST + kt
                    psc = pst((128, S), tag="p512")
                    nc.tensor.matmul(
                        psc[:], kT[hd, b * S + kt * 128:b * S + (kt + 1) * 128],
                        qT[hd, b * S:(b + 1) * S], start=True, stop=True)
                    et = sbt((128, S), tag="ET", bufs=5)
                    nc.scalar.activation(et[:], psc[:], EXP, bias=padb[:, g:g + 1],
                                         scale=0.125)
                    ET.append(et)
                for qt in range(ST):
                    pctx = pst((128, DK + 1), tag="p128")
                    for kt in range(ST):
                        nc.tensor.matmul(pctx[:],
                                         ET[kt][:, qt * 128:(qt + 1) * 128],
                                         vo[:, b * ST + kt, h, :],
                                         start=(kt == 0), stop=(kt == ST - 1))
                    rsr = sbt((128, 1), tag="rsr2", bufs=8)
                    nc.vector.reciprocal(rsr[:], pctx[:, DK:DK + 1])
                    nc.vector.tensor_scalar(ctx_sd[:, b * ST + qt, h * DK:(h + 1) * DK],
                                            pctx[:, 0:DK], rsr[:], None, OP.mult)
                    if emit_attn:
                        psc2 = pst((128, S), tag="p512")
                        nc.tensor.matmul(
                            psc2[:], qT[hd, b * S + qt * 128:b * S + (qt + 1) * 128],
                            kT[hd, b * S:(b + 1) * S], start=True, stop=True)
                        er = sbt((128, S), F32, tag="Eraw", bufs=2)
                        nc.scalar.activation(er[:], psc2[:], EXP, bias=0.0, scale=0.125)
                        at = sbt((128, S), F32, tag="attn", bufs=2)
                        nc.vector.scalar_tensor_tensor(at[:], er[:], rsr[:],
                                                       padmask[:, b, :],
                                                       OP.mult, OP.mult)
                        nc.sync.dma_start(
                            self.outs["attnp"][b, h, qt * 128:(qt + 1) * 128, :], at[:])

        # O-projection + residual + LN1
        ctx_ds = sbt((D, SW), tag="xds", bufs=3)
        for g in range(NT):
            ptr = pst((128, D), tag="p128")
            nc.tensor.transpose(ptr[:], ctx_sd[:, g, :], idr[:])
            cp(ctx_ds[:, g * 128:(g + 1) * 128], ptr[:])
        x1 = self.sb.tile([128, NT, D], F32, tag="x1", bufs=2)
        stats = self.sb.tile([128, NT, 2], F32, tag="stats", bufs=2)
        bno = self.sb.tile([128, NT, 6], F32, tag="bno", bufs=2)
        for g in range(NT):
            pao = pst((128, D), tag="p128")
            nc.tensor.matmul(pao[:], ctx_ds[:, g * 128:(g + 1) * 128], wo[:],
                             start=True, stop=True)
            nc.vector.scalar_tensor_tensor(x1[:, g, :], pao[:], 0.0, x_sd_l[:, g, :],
                                           OP.bypass, OP.add)
            nc.vector.bn_stats(bno[:, g, :], x1[:, g, :])
            nc.vector.bn_aggr(stats[:, g, :], bno[:, g, :])
        rstd = self._ln_rstd(stats)
        xn1 = sbt((128, NT, D), tag="xsd", bufs=3)
        for g in range(NT):
            nc.scalar.activation(xn1[:, g, :], x1[:, g, :], IDENT,
                                 bias=self._negmr(stats, rstd, g),
                                 scale=rstd[:, g:g + 1])
        xn1_ds = sbt((D, SW), tag="xds", bufs=3)
        for g in range(NT):
            ptr = pst((128, D), tag="p128")
            nc.tensor.transpose(ptr[:], xn1[:, g, :], idr[:])
            cp(xn1_ds[:, g * 128:(g + 1) * 128], ptr[:])

        # FF
        y1T = self.sb.tile([128, 4, SW], F32R, tag="y1T", bufs=2)
        for c in range(4):
            for sc in range(ST):
                pf = pst((128, 512), tag="p512")
                nc.tensor.matmul(pf[:], f1[:, c * 128:(c + 1) * 128],
                                 xn1_ds[:, sc * 512:(sc + 1) * 512],
                                 start=True, stop=True)
                nc.scalar.activation(y1T[:, c, sc * 512:(sc + 1) * 512], pf[:], RELU)
        sum_ds = sbt((D, SW), tag="xds", bufs=3)
        for sc in range(ST):
            sl = slice(sc * 512, (sc + 1) * 512)
            pf2 = pst((128, 512), tag="p512")
            for kt in range(4):
                nc.tensor.matmul(pf2[:], f2[:, kt, :], y1T[:, kt, sl],
                                 start=(kt == 0), stop=(kt == 3))
            nc.vector.tensor_tensor(sum_ds[:, sl], pf2[:], xn1_ds[:, sl], OP.add)
        x2 = self.sb.tile([128, NT, D], F32, tag="x1", bufs=2)
        stats2 = self.sb.tile([128, NT, 2], F32, tag="stats", bufs=2)
        bno2 = self.sb.tile([128, NT, 6], F32, tag="bno", bufs=2)
        for g in range(NT):
            ptr = pst((128, D), tag="p128")
            nc.tensor.transpose(ptr[:], sum_ds[:, g * 128:(g + 1) * 128], idr[:])
            cp(x2[:, g, :], ptr[:])
            nc.vector.bn_stats(bno2[:, g, :], x2[:, g, :])
            nc.vector.bn_aggr(stats2[:, g, :], bno2[:, g, :])
        rstd2 = self._ln_rstd(stats2)
        xn2 = sbt((128, NT, D), tag="xsd", bufs=3)
        for g in range(NT):
            nc.scalar.activation(xn2[:, g, :], x2[:, g, :], IDENT,
                                 bias=self._negmr(stats2, rstd2, g),
                                 scale=rstd2[:, g:g + 1])
        xn2_ds = sbt((D, SW), tag="xds", bufs=3)
        for g in range(NT):
            ptr = pst((128, D), tag="p128")
            nc.tensor.transpose(ptr[:], xn2[:, g, :], idr[:])
            cp(xn2_ds[:, g * 128:(g + 1) * 128], ptr[:])
        return xn2_ds, xn2

    def _ln_rstd(self, stats):
        nc, sbt = self.nc, self.sbt
        tmp = sbt((128, NT), F32, tag="lntmp", bufs=4)
        nc.vector.tensor_scalar(tmp[:], stats[:, :, 1], 1e-6, None, OP.add)
        sq = sbt((128, NT), F32, tag="lntmp", bufs=4)
        nc.scalar.activation(sq[:], tmp[:], SQRT)
        rstd = sbt((128, NT), F32, tag="lnrstd", bufs=2)
        nc.vector.reciprocal(rstd[:], sq[:])
        return rstd

    def _negmr(self, stats, rstd, g):
        nc = self.nc
        nm = self.sbt((128, 1), F32, tag="negmr", bufs=8)
        nc.vector.scalar_tensor_tensor(nm[:], stats[:, g, 0:1], -1.0,
                                       rstd[:, g:g + 1], OP.mult, OP.mult)
        return nm

    def emit_h(self, h_sd, h_ds, hout, cc_in, enc_idx):
        nc = self.nc
        for b in range(BL):
            nc.sync.dma_start(
                hout[b:b + 1, :].rearrange("one (t p c) -> p (one t) c", t=ST, p=128),
                h_sd[:, b * ST:(b + 1) * ST, :])
            for d in range(NC):
                nc.sync.dma_start(cc_in[d, enc_idx, b, :, :],
                                  h_ds[:, b * S + d * 64:b * S + (d + 1) * 64])


def kernel(**inputs):
    if "nc" not in _cached:
        _cached["nc"] = build()
    nc = _cached["nc"]

    f = lambda x: np.ascontiguousarray(np.asarray(x), dtype=np.float32)
    pep = np.asarray(inputs["peptide"])
    pe = _pe_np()

    base = {
        "emb": f(inputs["emb"]),
        "a_wq": f(inputs["a_wq"][0]), "a_wk": f(inputs["a_wk"][0]),
        "p1b": f(inputs["p1_b"]).reshape(8, 128).T.copy(),
        "p2w": f(inputs["p2_w"]),
        "p2b": f(inputs["p2_b"]).reshape(4, 128).T.copy(),
        "p3w": f(inputs["p3_w"]),
        "p3b": f(inputs["p3_b"]).reshape(1, 128),
        "peT": np.ascontiguousarray(pe.T),
        "ident32": np.eye(128, dtype=np.float32),
        "identr": np.eye(128, dtype=np.float32),
        "iota_v": np.tile(np.arange(V, dtype=np.float32), (128, 1)),
        "iota_row": np.arange(S, dtype=np.float32).reshape(1, S),
        "iota_part": np.ascontiguousarray(
            np.arange(128, dtype=np.float32)[:, None]
            + 128.0 * np.arange(ST, dtype=np.float32)[None, :]),
        "ones_r": np.ones((128, 128), dtype=np.float32),
    }
    for p in ("e", "t"):
        for w in ("wq", "wk", "wv", "wo", "f1", "f2"):
            base[f"{p}_{w}"] = f(inputs[f"{p}_{w}"])
    p1w = f(inputs["p1_w"])
    in_maps = []
    for c in range(NC):
        m = dict(base)
        m["pep_f"] = pep[c * BL:(c + 1) * BL].astype(np.float32)
        m["p1s"] = np.ascontiguousarray(p1w[c * KSH:(c + 1) * KSH, :])
        in_maps.append(m)

    res = run_bass_kernel_spmd(nc, in_maps, core_ids=list(range(NC)))
    rs = res.results
    h1 = np.concatenate([rs[c]["h1p"] for c in range(NC)], axis=0)
    h2 = np.concatenate([rs[c]["h2p"] for c in range(NC)], axis=0)
    attn = np.concatenate([rs[c]["attnp"] for c in range(NC)], axis=0)
    kernel._debug = {
        "mask": np.concatenate([rs[c]["maskdbg"] for c in range(NC)]),
        "tok": np.concatenate([rs[c]["tokdbg"] for c in range(NC)]),
        "res": res,
    }
    return (h1, rs[0]["z1p"], h2, rs[0]["z2p"], attn)


def _pe_np():
    import math
    pos = np.arange(S, dtype=np.float32)[:, None]
    div = np.exp(np.arange(0, D, 2, dtype=np.float32) * (-math.log(10000.0) / D))
    pe = np.zeros((S, D), np.float32)
    pe[:, 0::2] = np.sin(pos * div)
    pe[:, 1::2] = np.cos(pos * div)
    return pe


# revision 2
# speedup vs baseline: 1.1867x; 1.1867x over previous
"""TRN2 Bass kernel for nn_CBBIOMFP_9234179686614 (topk_masking).

Reference computation (B=32, S=512, D=128, V=25, H=2):
  x = emb[peptide] + pe
  a-encoder (1 layer; only QK needed) -> attention col-sums -> mask [B,S]
  creat_mask: per row drop the ceil(0.2*j) smallest mask values before the
  first pad (token==0) position j -> mask_pep
  h  = e-encoder(3 layers)(x),  attn_score = layer-3 attention probs
  hm = t-encoder(2 layers)(emb[mask_pep]+pe)
  z1 = head(h.flat), z2 = head(hm.flat)   (65536->1024->512->128 MLP)
  returns (h1, z1, h2, z2, attn_score)

Sharding: data-parallel over batch for the encoders (4 seqs/core); the big
65536x1024 head matmul is k-sharded (each core owns 8192 contraction rows
of p1_w), using an on-device AllToAll to redistribute h and an AllReduce
for the layer-1 partials.  Matmuls use float32r where the moving dim is
>=256; the top-k ranking path is kept in exact fp32.
"""
import sys
import numpy as np

sys.path.insert(0, "/opt/trn_rl_repo")

import concourse.bass as bass
import concourse.mybir as mybir
import concourse.tile as tile
from concourse import bacc
from concourse.bass_utils import run_bass_kernel_spmd

F32 = mybir.dt.float32
F32R = mybir.dt.float32r
EXP = mybir.ActivationFunctionType.Exp
RELU = mybir.ActivationFunctionType.Relu
SQRT = mybir.ActivationFunctionType.Sqrt
IDENT = mybir.ActivationFunctionType.Identity
OP = mybir.AluOpType
AX = mybir.AxisListType

B, S, D, V, H = 32, 512, 128, 25, 2
DK = D // H          # 64
DFF = 4 * D          # 512
NC = 8               # cores
BL = B // NC         # 4 seqs per core
ST = S // 128        # 4 s-tiles per seq
NT = BL * ST         # 16 s-tiles per core
SW = BL * S          # 2048 tokens per core
KSH = S // NC * D    # 8192 head contraction rows per core
PRECET = float(np.float32(0.2))

_cached = {}


def build():
    nc = bacc.Bacc(None, num_devices=NC)

    din = {}

    def inp(name, shape, dt=F32R):
        din[name] = nc.dram_tensor(name, list(shape), dt, kind="ExternalInput")
        return din[name]

    inp("pep_f", (BL, S), F32)
    inp("emb", (V, D))
    inp("a_wq", (D, D)); inp("a_wk", (D, D))
    for p, L in (("e", 3), ("t", 2)):
        inp(p + "_wq", (L, D, D)); inp(p + "_wk", (L, D, D))
        inp(p + "_wv", (L, D, D)); inp(p + "_wo", (L, D, D))
        inp(p + "_f1", (L, D, DFF)); inp(p + "_f2", (L, DFF, D))
    inp("p1s", (KSH, 1024))
    inp("p1b", (128, 8), F32)
    inp("p2w", (1024, 512))
    inp("p2b", (128, 4), F32)
    inp("p3w", (512, D))
    inp("p3b", (1, D))
    inp("peT", (D, S), F32)
    inp("ident32", (128, 128), F32)
    inp("identr", (128, 128))
    inp("iota_v", (128, V), F32)
    inp("iota_row", (1, S), F32)
    inp("iota_part", (128, ST), F32)
    inp("ones_r", (128, 128))

    outs = {
        "h1p": nc.dram_tensor("h1p", [BL, S * D], F32, kind="ExternalOutput"),
        "h2p": nc.dram_tensor("h2p", [BL, S * D], F32, kind="ExternalOutput"),
        "attnp": nc.dram_tensor("attnp", [BL, H, S, S], F32, kind="ExternalOutput"),
        "z1p": nc.dram_tensor("z1p", [B, D], F32, kind="ExternalOutput"),
        "z2p": nc.dram_tensor("z2p", [B, D], F32, kind="ExternalOutput"),
        "maskdbg": nc.dram_tensor("maskdbg", [BL, S], F32, kind="ExternalOutput"),
        "tokdbg": nc.dram_tensor("tokdbg", [BL, S], F32, kind="ExternalOutput"),
    }

    with tile.TileContext(nc) as tc:
        Body(nc, tc, din, outs).run()
    nc.compile()
    return nc


class Body:
    def __init__(self, nc, tc, din, outs):
        self.nc, self.tc, self.din, self.outs = nc, tc, din, outs

    def run(self):
        nc, tc, din = self.nc, self.tc, self.din
        import contextlib
        with contextlib.ExitStack() as es:
            self.sb = es.enter_context(tc.tile_pool(name="sb", bufs=1))
            self.ps = es.enter_context(tc.tile_pool(name="ps", bufs=1, space="PSUM"))
            self.dram = es.enter_context(tc.tile_pool(name="dram", bufs=1, space="DRAM"))
            self._body()

    def sbt(self, shape, dt=F32R, tag=None, bufs=1):
        return self.sb.tile(list(shape), dt, tag=tag, bufs=bufs)

    def pst(self, shape, tag, bufs=2):
        return self.ps.tile(list(shape), F32, tag=tag, bufs=bufs)

    def _body(self):
        nc, din, outs = self.nc, self.din, self.outs
        sbt, pst = self.sbt, self.pst
        cp = nc.vector.tensor_copy

        # ---- constants ----
        emb_sb = sbt((V, D), tag="c_emb")
        nc.sync.dma_start(emb_sb[:], din["emb"][:])
        peT = sbt((D, S), F32, tag="c_peT")
        nc.sync.dma_start(peT[:], din["peT"][:])
        id32 = sbt((128, 128), F32, tag="c_id32")
        nc.sync.dma_start(id32[:], din["ident32"][:])
        idr = sbt((128, 128), tag="c_idr")
        nc.sync.dma_start(idr[:], din["identr"][:])
        iov = sbt((128, V), F32, tag="c_iov")
        nc.sync.dma_start(iov[:], din["iota_v"][:])
        ior = sbt((1, S), F32, tag="c_ior")
        nc.sync.dma_start(ior[:], din["iota_row"][:])
        iop = sbt((128, ST), F32, tag="c_iop")
        nc.sync.dma_start(iop[:], din["iota_part"][:])
        onesr = sbt((128, 128), tag="c_onesr")
        nc.sync.dma_start(onesr[:], din["ones_r"][:])
        pep = sbt((BL, S), F32, tag="c_pep")
        nc.sync.dma_start(pep[:], din["pep_f"][:])
        self.idr, self.id32, self.onesr = idr, id32, onesr

        # ---- embedding of original tokens ----
        tokp = sbt((128, NT), F32, tag="tokp")
        padbias = sbt((128, NT), F32, tag="padb")
        padmask = sbt((128, BL, S), F32, tag="padm")
        ohT = sbt((V, SW), tag="ohT", bufs=2)
        x_ds = sbt((D, SW), tag="xds", bufs=3)
        x_sd = sbt((128, NT, D), tag="xsd", bufs=3)

        def embed(tok_col_src, ohT_o, xds_o, xsd_o, padb_o):
            for b in range(BL):
                for t in range(ST):
                    g = b * ST + t
                    tcol = tok_col_src(b, t, g)
                    nc.vector.tensor_scalar(padb_o[:, g:g + 1], tcol, 0.0, -1e9,
                                            OP.is_equal, OP.mult)
                    oh = sbt((128, V), tag="oh", bufs=3)
                    nc.vector.tensor_scalar(oh[:], iov[:], tcol, None, OP.is_equal)
                    poh = pst((V, 128), tag="p1c")
                    nc.tensor.transpose(poh[:], oh[:], idr[:])
                    cp(ohT_o[:, b * S + t * 128:b * S + (t + 1) * 128], poh[:])
                px = pst((D, S), tag="p512")
                nc.tensor.matmul(px[:], emb_sb[:], ohT_o[:, b * S:(b + 1) * S],
                                 start=True, stop=True)
                nc.vector.tensor_tensor(xds_o[:, b * S:(b + 1) * S], px[:], peT[:], OP.add)
                for t in range(ST):
                    pxs = pst((128, D), tag="p128")
                    nc.tensor.transpose(
                        pxs[:], xds_o[:, b * S + t * 128:b * S + (t + 1) * 128], idr[:])
                    cp(xsd_o[:, b * ST + t, :], pxs[:])

        def tok_from_pep(b, t, g):
            ptok = pst((128, 1), tag="p1c")
            nc.tensor.transpose(ptok[:], pep[b:b + 1, t * 128:(t + 1) * 128]
                                .bitcast(F32R), idr[:])
            cp(tokp[:, g:g + 1], ptok[:])
            return tokp[:, g:g + 1]

        embed(tok_from_pep, ohT, x_ds, x_sd, padbias)

        # pad masks broadcast across partitions (exact fp32 copy)
        padrow = sbt((1, BL, S), F32, tag="padrow")
        for b in range(BL):
            nc.vector.tensor_scalar(padrow[:, b, :], pep[b:b + 1, :], 0.0, 1.0,
                                    OP.not_equal, OP.mult)
            nc.gpsimd.partition_broadcast(padmask[:, b, :], padrow[:, b, :])

        # ---- a-encoder: E -> col-sums -> mask ----
        awq = sbt((D, D), tag="wq", bufs=2)
        nc.sync.dma_start(awq[:], din["a_wq"][:])
        awk = sbt((D, D), tag="wk", bufs=2)
        nc.sync.dma_start(awk[:], din["a_wk"][:])
        qT = sbt((D, SW), tag="qT", bufs=2)
        kT = sbt((D, SW), tag="kT", bufs=2)
        for c in range(ST):
            sl = slice(c * 512, (c + 1) * 512)
            pq = pst((D, 512), tag="p512")
            nc.tensor.matmul(pq[:], awq[:], x_ds[:, sl], start=True, stop=True)
            cp(qT[:, sl], pq[:])
            pk = pst((D, 512), tag="p512")
            nc.tensor.matmul(pk[:], awk[:], x_ds[:, sl], start=True, stop=True)
            cp(kT[:, sl], pk[:])

        mask_sb = sbt((1, BL, S), F32, tag="maskrow")
        for b in range(BL):
            pmask = pst((1, S), tag="prow", bufs=1)
            n_mm = 0
            for h in range(H):
                hd = slice(h * DK, (h + 1) * DK)
                for qt in range(ST):
                    psc = pst((128, S), tag="p512")
                    nc.tensor.matmul(
                        psc[:], qT[hd, b * S + qt * 128:b * S + (qt + 1) * 128],
                        kT[hd, b * S:(b + 1) * S], start=True, stop=True)
                    e_raw = sbt((128, S), F32, tag="Eraw", bufs=2)
                    nc.scalar.activation(e_raw[:], psc[:], EXP, bias=0.0, scale=0.125)
                    em = sbt((128, S), F32, tag="EM", bufs=2)
                    rs = sbt((128, 1), F32, tag="rs", bufs=8)
                    nc.vector.scalar_tensor_tensor(em[:], e_raw[:], 1.0,
                                                   padmask[:, b, :], OP.bypass,
                                                   OP.mult, accum_out=rs[:])
                    rsr = sbt((128, 1), F32, tag="rsr", bufs=8)
                    nc.vector.reciprocal(rsr[:], rs[:])
                    nc.tensor.matmul(pmask[:], rsr[:], em[:],
                                     start=(n_mm == 0), stop=(n_mm == 2 * ST - 1))
                    n_mm += 1
            cp(mask_sb[:, b, :], pmask[:])
            nc.sync.dma_start(outs["maskdbg"][b:b + 1, :], mask_sb[:, b, :])

        # ---- creat_mask + masked-token embedding ----
        tokp2 = sbt((128, NT), F32, tag="tokp2")
        padbias2 = sbt((128, NT), F32, tag="padb2")
        ohT2 = sbt((V, SW), tag="ohT", bufs=2)
        xm_ds = sbt((D, SW), tag="xmds")
        xm_sd = sbt((128, NT, D), tag="xmsd")

        for b in range(BL):
            mrow = mask_sb[:, b, :]
            is0 = sbt((1, S), F32, tag="cmrow", bufs=6)
            nc.vector.tensor_scalar(is0[:], pep[b:b + 1, :], 0.0, None, OP.is_equal)
            tmp = sbt((1, S), F32, tag="cmrow", bufs=6)
            nc.vector.scalar_tensor_tensor(tmp[:], is0[:], -1e9, ior[:], OP.mult, OP.add)
            jb = sbt((1, 1), F32, tag="cmsc", bufs=8)
            nc.vector.tensor_reduce(jb[:], tmp[:], AX.X, OP.min)
            h0 = sbt((1, 1), F32, tag="cmsc", bufs=8)
            nc.vector.tensor_scalar(h0[:], jb[:], 0.0, None, OP.is_lt)
            jv = sbt((1, 1), F32, tag="cmsc", bufs=8)
            nc.vector.scalar_tensor_tensor(jv[:], h0[:], 1e9, jb[:], OP.mult, OP.add)
            th = sbt((1, 1), F32, tag="cmsc", bufs=8)
            nc.vector.tensor_scalar(th[:], jv[:], PRECET, None, OP.mult)
            valid = sbt((1, S), F32, tag="cmrow", bufs=6)
            nc.vector.tensor_scalar(valid[:], ior[:], jv[:], None, OP.is_lt)
            mv = sbt((1, S), F32, tag="cmrow", bufs=6)
            nc.vector.scalar_tensor_tensor(mv[:], valid[:], -1e30, mrow, OP.mult, OP.add)
            mv2 = sbt((1, S), F32, tag="cmrow", bufs=6)
            nc.vector.tensor_scalar(mv2[:], mv[:], 1e30, None, OP.add)
            # exact fp32 partition broadcasts
            mvb = sbt((128, S), F32, tag="mvb")
            nc.gpsimd.partition_broadcast(mvb[:], mv2[:])
            tj = sbt((1, 2), F32, tag="cmsc2", bufs=2)
            cp(tj[:, 0:1], th[:])
            cp(tj[:, 1:2], jv[:])
            tjb = sbt((128, 2), F32, tag="tjb", bufs=2)
            nc.gpsimd.partition_broadcast(tjb[:], tj[:])
            for t in range(ST):
                g = b * ST + t
                pmp = pst((128, 1), tag="p1c")
                nc.tensor.transpose(pmp[:], mrow[:, t * 128:(t + 1) * 128], id32[:])
                mpart = sbt((128, 1), F32, tag="mpart", bufs=4)
                cp(mpart[:], pmp[:])
                scr = sbt((128, S), F32, tag="cmscr")
                cnt = sbt((128, 1), F32, tag="cnt", bufs=8)
                nc.vector.tensor_scalar(scr[:], mvb[:], mpart[:], None, OP.is_lt,
                                        accum_out=cnt[:])
                ge1 = sbt((128, 1), F32, tag="cnt", bufs=8)
                nc.vector.tensor_scalar(ge1[:], cnt[:], tjb[:, 0:1], None, OP.is_ge)
                keep = sbt((128, 1), F32, tag="cnt", bufs=8)
                nc.vector.scalar_tensor_tensor(keep[:], iop[:, t:t + 1], tjb[:, 1:2],
                                               ge1[:], OP.is_ge, OP.logical_or)
                nc.vector.tensor_tensor(tokp2[:, g:g + 1], tokp[:, g:g + 1], keep[:],
                                        OP.mult)
            nc.sync.dma_start(
                outs["tokdbg"][b:b + 1, :].rearrange("one (t p) -> p (one t)", p=128),
                tokp2[:, b * ST:(b + 1) * ST])

        def tok_from_masked(b, t, g):
            return tokp2[:, g:g + 1]

        embed(tok_from_masked, ohT2, xm_ds, xm_sd, padbias2)

        # ---- full encoder layers ----
        cc_in = self.dram.tile([NC, 2, BL, D, S // NC], F32)

        cur_ds, cur_sd = x_ds, x_sd
        for l in range(3):
            cur_ds, cur_sd = self.enc_layer(
                cur_ds, cur_sd, "e", l, padbias, padmask, emit_attn=(l == 2))
        self.emit_h(cur_sd, cur_ds, outs["h1p"], cc_in, 0)

        cur_ds, cur_sd = xm_ds, xm_sd
        for l in range(2):
            cur_ds, cur_sd = self.enc_layer(
                cur_ds, cur_sd, "t", l, padbias2, None, emit_attn=False)
        self.emit_h(cur_sd, cur_ds, outs["h2p"], cc_in, 1)

        # ---- AllToAll + head ----
        import os
        if os.environ.get("SKIP_HEAD"):
            return
        cc_out = self.dram.tile([NC, 2, BL, D, S // NC], F32)
        nc.gpsimd.collective_compute("AllToAll", OP.bypass,
                                     replica_groups=[list(range(NC))],
                                     ins=[cc_in.opt()], outs=[cc_out.opt()])

        py = pst((64, 1024), tag="phead", bufs=1)
        for sl in range(64):
            lh = sbt((128, 64), tag="hlhs", bufs=4)
            nc.sync.dma_start(
                lh[:].rearrange("d (e c b) -> d e c b", e=2, c=NC, b=BL),
                cc_out[:, :, :, :, sl].rearrange("c e b d -> d e c b"))
            w1 = sbt((128, 1024), tag="w1", bufs=3)
            nc.sync.dma_start(w1[:], din["p1s"][sl * 128:(sl + 1) * 128, :])
            nc.tensor.matmul(py[:, 0:512], lh[:].bitcast(F32R), w1[:, 0:512],
                             start=(sl == 0), stop=(sl == 63), skip_group_check=True)
            nc.tensor.matmul(py[:, 512:1024], lh[:].bitcast(F32R), w1[:, 512:1024],
                             start=(sl == 0), stop=(sl == 63), skip_group_check=True)
        y1_sb = sbt((64, 1024), F32, tag="y1sb", bufs=2)
        cp(y1_sb[:], py[:])
        y1in = self.dram.tile([64, 1024], F32)
        y1out = self.dram.tile([64, 1024], F32)
        nc.sync.dma_start(y1in[:], y1_sb[:])
        nc.gpsimd.collective_compute("AllReduce", OP.add,
                                     replica_groups=[list(range(NC))],
                                     ins=[y1in.opt()], outs=[y1out.opt()])
        y1f = sbt((64, 1024), F32, tag="y1sb", bufs=2)
        nc.sync.dma_start(y1f[:], y1out[:])

        p1b_sb = sbt((128, 8), F32, tag="p1b")
        nc.sync.dma_start(p1b_sb[:], din["p1b"][:])
        p2b_sb = sbt((128, 4), F32, tag="p2b")
        nc.sync.dma_start(p2b_sb[:], din["p2b"][:])
        p2w_sb = self.sb.tile([128, 8, 512], F32R, tag="y1T", bufs=2)
        nc.sync.dma_start(p2w_sb[:], din["p2w"][:].rearrange("(k p) n -> p k n", p=128))
        p3w_sb = sbt((128, 4, D), tag="p3w")
        nc.sync.dma_start(p3w_sb[:], din["p3w"][:].rearrange("(k p) n -> p k n", p=128))
        p3b_sb = sbt((1, D), tag="p3b")
        nc.sync.dma_start(p3b_sb[:], din["p3b"][:])

        y1T_h = sbt((128, 8, 64), tag="y1Th")
        for kt in range(8):
            pt = pst((128, 64), tag="p128")
            nc.tensor.transpose(pt[:], y1f[:, kt * 128:(kt + 1) * 128].bitcast(F32R),
                                idr[:])
            nc.scalar.activation(y1T_h[:, kt, :], pt[:], RELU,
                                 bias=p1b_sb[:, kt:kt + 1])
        pz2 = pst((64, 512), tag="phead", bufs=1)
        for kt in range(8):
            nc.tensor.matmul(pz2[:], y1T_h[:, kt, :], p2w_sb[:, kt, :],
                             start=(kt == 0), stop=(kt == 7))
        y2_sb = sbt((64, 512), F32, tag="y2sb")
        cp(y2_sb[:], pz2[:])
        y2T_h = sbt((128, 4, 64), tag="y2Th")
        for kt in range(4):
            pt = pst((128, 64), tag="p128")
            nc.tensor.transpose(pt[:], y2_sb[:, kt * 128:(kt + 1) * 128].bitcast(F32R),
                                idr[:])
            nc.scalar.activation(y2T_h[:, kt, :], pt[:], RELU,
                                 bias=p2b_sb[:, kt:kt + 1])
        pz3 = pst((64, D), tag="p128")
        for kt in range(4):
            nc.tensor.matmul(pz3[:], y2T_h[:, kt, :], p3w_sb[:, kt, :],
                             start=(kt == 0), stop=False)
        nc.tensor.matmul(pz3[:], onesr[0:1, 0:64], p3b_sb[:], start=False, stop=True)
        z_sb = sbt((64, D), F32, tag="zsb")
        cp(z_sb[:], pz3[:])
        nc.sync.dma_start(outs["z1p"][:], z_sb[0:32, :])
        nc.sync.dma_start(outs["z2p"][:], z_sb[32:64, :])

    # ------------------------------------------------------------------
    def enc_layer(self, x_ds_l, x_sd_l, pre, l, padb, padmask, emit_attn):
        nc, din = self.nc, self.din
        sbt, pst = self.sbt, self.pst
        cp = nc.vector.tensor_copy
        idr = self.idr

        wq = sbt((D, D), tag="wq", bufs=2)
        nc.sync.dma_start(wq[:], din[pre + "_wq"][l])
        wk = sbt((D, D), tag="wk", bufs=2)
        nc.sync.dma_start(wk[:], din[pre + "_wk"][l])
        wv = sbt((D, D), tag="wv", bufs=2)
        nc.sync.dma_start(wv[:], din[pre + "_wv"][l])
        wo = sbt((D, D), tag="wo", bufs=2)
        nc.sync.dma_start(wo[:], din[pre + "_wo"][l])
        f1 = sbt((D, DFF), tag="f1", bufs=2)
        nc.sync.dma_start(f1[:], din[pre + "_f1"][l])
        f2 = self.sb.tile([128, 4, D], F32R, tag="f2", bufs=2)
        nc.sync.dma_start(f2[:], din[pre + "_f2"][l].rearrange("(k p) d -> p k d", p=128))

        qT = sbt((D, SW), tag="qT", bufs=2)
        kT = sbt((D, SW), tag="kT", bufs=2)
        for c in range(ST):
            sl = slice(c * 512, (c + 1) * 512)
            pq = pst((D, 512), tag="p512")
            nc.tensor.matmul(pq[:], wq[:], x_ds_l[:, sl], start=True, stop=True)
            cp(qT[:, sl], pq[:])
            pk = pst((D, 512), tag="p512")
            nc.tensor.matmul(pk[:], wk[:], x_ds_l[:, sl], start=True, stop=True)
            cp(kT[:, sl], pk[:])

        vo = self.sb.tile([128, NT, H, DK + 1], F32R, tag="vones", bufs=1)
        nc.vector.memset(vo[:, :, :, DK:DK + 1], 1.0)
        for g in range(NT):
            pv = pst((128, D), tag="p128")
            nc.tensor.matmul(pv[:], x_ds_l[:, g * 128:(g + 1) * 128], wv[:],
                             start=True, stop=True)
            for h in range(H):
                cp(vo[:, g, h, 0:DK], pv[:, h * DK:(h + 1) * DK])

        ctx_sd = sbt((128, NT, D), tag="xsd", bufs=3)
        for b in range(BL):
            for h in range(H):
                hd = slice(h * DK, (h + 1) * DK)
                ET = []
                for kt in range(ST):
                    g = b * ST + kt
                    psc = pst((128, S), tag="p512")
                    nc.tensor.matmul(
                        psc[:], kT[hd, b * S + kt * 128:b * S + (kt + 1) * 128],
                        qT[hd, b * S:(b + 1) * S], start=True, stop=True)
                    et = sbt((128, S), tag="ET", bufs=5)
                    nc.scalar.activation(et[:], psc[:], EXP, bias=padb[:, g:g + 1],
                                         scale=0.125)
                    ET.append(et)
                for qt in range(ST):
                    pctx = pst((128, DK + 1), tag="p128")
                    for kt in range(ST):
                        nc.tensor.matmul(pctx[:],
                                         ET[kt][:, qt * 128:(qt + 1) * 128],
                                         vo[:, b * ST + kt, h, :],
                                         start=(kt == 0), stop=(kt == ST - 1))
                    rsr = sbt((128, 1), tag="rsr2", bufs=8)
                    nc.vector.reciprocal(rsr[:], pctx[:, DK:DK + 1])
                    nc.vector.tensor_scalar(ctx_sd[:, b * ST + qt, h * DK:(h + 1) * DK],
                                            pctx[:, 0:DK], rsr[:], None, OP.mult)
                    if emit_attn:
                        import os
                        if os.environ.get("SKIP_ATTN_OUT"):
                            continue
                        psc2 = pst((128, S), tag="p512")
                        nc.tensor.matmul(
                            psc2[:], qT[hd, b * S + qt * 128:b * S + (qt + 1) * 128],
                            kT[hd, b * S:(b + 1) * S], start=True, stop=True)
                        er = sbt((128, S), F32, tag="Eraw", bufs=2)
                        nc.scalar.activation(er[:], psc2[:], EXP, bias=0.0, scale=0.125)
                        at = sbt((128, S), F32, tag="attn", bufs=2)
                        nc.vector.scalar_tensor_tensor(at[:], er[:], rsr[:],
                                                       padmask[:, b, :],
                                                       OP.mult, OP.mult)
                        nc.sync.dma_start(
                            self.outs["attnp"][b, h, qt * 128:(qt + 1) * 128, :], at[:])

        # O-projection + residual + LN1
        ctx_ds = sbt((D, SW), tag="xds", bufs=3)
        for g in range(NT):
            ptr = pst((128, D), tag="p128")
            nc.tensor.transpose(ptr[:], ctx_sd[:, g, :], idr[:])
            cp(ctx_ds[:, g * 128:(g + 1) * 128], ptr[:])
        x1 = self.sb.tile([128, NT, D], F32, tag="x1", bufs=2)
        stats = self.sb.tile([128, NT, 2], F32, tag="stats", bufs=2)
        bno = self.sb.tile([128, NT, 6], F32, tag="bno", bufs=2)
        for g in range(NT):
            pao = pst((128, D), tag="p128")
            nc.tensor.matmul(pao[:], ctx_ds[:, g * 128:(g + 1) * 128], wo[:],
                             start=True, stop=True)
            nc.vector.scalar_tensor_tensor(x1[:, g, :], pao[:], 0.0, x_sd_l[:, g, :],
                                           OP.bypass, OP.add)
            nc.vector.bn_stats(bno[:, g, :], x1[:, g, :])
            nc.vector.bn_aggr(stats[:, g, :], bno[:, g, :])
        rstd = self._ln_rstd(stats)
        xn1 = sbt((128, NT, D), tag="xsd", bufs=3)
        for g in range(NT):
            nc.scalar.activation(xn1[:, g, :], x1[:, g, :], IDENT,
                                 bias=self._negmr(stats, rstd, g),
                                 scale=rstd[:, g:g + 1])
        xn1_ds = sbt((D, SW), tag="xds", bufs=3)
        for g in range(NT):
            ptr = pst((128, D), tag="p128")
            nc.tensor.transpose(ptr[:], xn1[:, g, :], idr[:])
            cp(xn1_ds[:, g * 128:(g + 1) * 128], ptr[:])

        # FF
        y1T = self.sb.tile([128, 4, SW], F32R, tag="y1T", bufs=2)
        for c in range(4):
            for sc in range(ST):
                pf = pst((128, 512), tag="p512")
                nc.tensor.matmul(pf[:], f1[:, c * 128:(c + 1) * 128],
                                 xn1_ds[:, sc * 512:(sc + 1) * 512],
                                 start=True, stop=True)
                nc.scalar.activation(y1T[:, c, sc * 512:(sc + 1) * 512], pf[:], RELU)
        sum_ds = sbt((D, SW), tag="xds", bufs=3)
        for sc in range(ST):
            sl = slice(sc * 512, (sc + 1) * 512)
            pf2 = pst((128, 512), tag="p512")
            for kt in range(4):
                nc.tensor.matmul(pf2[:], f2[:, kt, :], y1T[:, kt, sl],
                                 start=(kt == 0), stop=(kt == 3))
            nc.vector.tensor_tensor(sum_ds[:, sl], pf2[:], xn1_ds[:, sl], OP.add)
        x2 = self.sb.tile([128, NT, D], F32, tag="x1", bufs=2)
        stats2 = self.sb.tile([128, NT, 2], F32, tag="stats", bufs=2)
        bno2 = self.sb.tile([128, NT, 6], F32, tag="bno", bufs=2)
        for g in range(NT):
            ptr = pst((128, D), tag="p128")
            nc.tensor.transpose(ptr[:], sum_ds[:, g * 128:(g + 1) * 128], idr[:])
            cp(x2[:, g, :], ptr[:])
            nc.vector.bn_stats(bno2[:, g, :], x2[:, g, :])
            nc.vector.bn_aggr(stats2[:, g, :], bno2[:, g, :])
        rstd2 = self._ln_rstd(stats2)
        xn2 = sbt((128, NT, D), tag="xsd", bufs=3)
        for g in range(NT):
            nc.scalar.activation(xn2[:, g, :], x2[:, g, :], IDENT,
                                 bias=self._negmr(stats2, rstd2, g),
                                 scale=rstd2[:, g:g + 1])
        xn2_ds = sbt((D, SW), tag="xds", bufs=3)
        for g in range(NT):
            ptr = pst((128, D), tag="p128")
            nc.tensor.transpose(ptr[:], xn2[:, g, :], idr[:])
            cp(xn2_ds[:, g * 128:(g + 1) * 128], ptr[:])
        return xn2_ds, xn2

    def _ln_rstd(self, stats):
        nc, sbt = self.nc, self.sbt
        tmp = sbt((128, NT), F32, tag="lntmp", bufs=4)
        nc.vector.tensor_scalar(tmp[:], stats[:, :, 1], 1e-6, None, OP.add)
        sq = sbt((128, NT), F32, tag="lntmp", bufs=4)
        nc.scalar.activation(sq[:], tmp[:], SQRT)
        rstd = sbt((128, NT), F32, tag="lnrstd", bufs=2)
        nc.vector.reciprocal(rstd[:], sq[:])
        return rstd

    def _negmr(self, stats, rstd, g):
        nc = self.nc
        nm = self.sbt((128, 1), F32, tag="negmr", bufs=8)
        nc.vector.scalar_tensor_tensor(nm[:], stats[:, g, 0:1], -1.0,
                                       rstd[:, g:g + 1], OP.mult, OP.mult)
        return nm

    def emit_h(self, h_sd, h_ds, hout, cc_in, enc_idx):
        nc = self.nc
        for b in range(BL):
            nc.sync.dma_start(
                hout[b:b + 1, :].rearrange("one (t p c) -> p (one t) c", t=ST, p=128),
                h_sd[:, b * ST:(b + 1) * ST, :])
            for d in range(NC):
                nc.sync.dma_start(cc_in[d, enc_idx, b, :, :],
                                  h_ds[:, b * S + d * 64:b * S + (d + 1) * 64])


def kernel(**inputs):
    if "nc" not in _cached:
        _cached["nc"] = build()
    nc = _cached["nc"]

    f = lambda x: np.ascontiguousarray(np.asarray(x), dtype=np.float32)
    pep = np.asarray(inputs["peptide"])
    pe = _pe_np()

    base = {
        "emb": f(inputs["emb"]),
        "a_wq": f(inputs["a_wq"][0]), "a_wk": f(inputs["a_wk"][0]),
        "p1b": f(inputs["p1_b"]).reshape(8, 128).T.copy(),
        "p2w": f(inputs["p2_w"]),
        "p2b": f(inputs["p2_b"]).reshape(4, 128).T.copy(),
        "p3w": f(inputs["p3_w"]),
        "p3b": f(inputs["p3_b"]).reshape(1, 128),
        "peT": np.ascontiguousarray(pe.T),
        "ident32": np.eye(128, dtype=np.float32),
        "identr": np.eye(128, dtype=np.float32),
        "iota_v": np.tile(np.arange(V, dtype=np.float32), (128, 1)),
        "iota_row": np.arange(S, dtype=np.float32).reshape(1, S),
        "iota_part": np.ascontiguousarray(
            np.arange(128, dtype=np.float32)[:, None]
            + 128.0 * np.arange(ST, dtype=np.float32)[None, :]),
        "ones_r": np.ones((128, 128), dtype=np.float32),
    }
    for p in ("e", "t"):
        for w in ("wq", "wk", "wv", "wo", "f1", "f2"):
            base[f"{p}_{w}"] = f(inputs[f"{p}_{w}"])
    p1w = f(inputs["p1_w"])
    in_maps = []
    for c in range(NC):
        m = dict(base)
        m["pep_f"] = pep[c * BL:(c + 1) * BL].astype(np.float32)
        m["p1s"] = np.ascontiguousarray(p1w[c * KSH:(c + 1) * KSH, :])
        in_maps.append(m)

    res = run_bass_kernel_spmd(nc, in_maps, core_ids=list(range(NC)))
    rs = res.results
    h1 = np.concatenate([rs[c]["h1p"] for c in range(NC)], axis=0)
    h2 = np.concatenate([rs[c]["h2p"] for c in range(NC)], axis=0)
    attn = np.concatenate([rs[c]["attnp"] for c in range(NC)], axis=0)
    kernel._debug = {
        "mask": np.concatenate([rs[c]["maskdbg"] for c in range(NC)]),
        "tok": np.concatenate([rs[c]["tokdbg"] for c in range(NC)]),
        "res": res,
    }
    return (h1, rs[0]["z1p"], h2, rs[0]["z2p"], attn)


def _pe_np():
    import math
    pos = np.arange(S, dtype=np.float32)[:, None]
    div = np.exp(np.arange(0, D, 2, dtype=np.float32) * (-math.log(10000.0) / D))
    pe = np.zeros((S, D), np.float32)
    pe[:, 0::2] = np.sin(pos * div)
    pe[:, 1::2] = np.cos(pos * div)
    return pe
